# revision 75
# baseline (speedup 1.0000x reference)
"""Multi-head attention (B=2, S=2048, E=768, H=12, D=64) on 8 trn2 NeuronCores.

Sharding: tensor-parallel over heads x data-parallel over batch.
Core c handles batch b = c // 4 and heads {3g, 3g+1, 3g+2} for g = c % 4.
Each core computes the QKV projection for its 3 heads over the full
sequence, causal attention, and a partial output projection
(w_out sliced along its input dim).  The host sums the 4 partial outputs
per batch (the unshard required by input-dim-sharded w_out).

Device inputs (per core, SPMD - same program, different data; x / qkv
weights / mask / y ship as bf16 - halves DMA bytes at the same
1 cycle/row matmul rate, ~0.4% rms error vs the 2e-2 budget):
  xT     [4, 768, 512]  x[b] transposed, seq-block-major so every DMA is
                        one fully-contiguous block (2x effective DMA rate
                        vs strided [E, S] slices)
  wqkvT  [768, 384]     q/k weight slices, transposed; column order:
                        [q_h0|q_h1 (128), k_h0|k_h1 (128), q2 (64), k2 (64)]
  wvnat  [768, 192]     v weights for the 3 heads (bf16 needs no f32r
                        256-col padding)
  woutT  [256, 768]     w_out.T rows for this core's features (f32r)
  maskb  [128, 896]     sliding causal mask: maskb[kp, t] = 1 iff kp <= t-384
  bqkv   [128, 10]      per-pass bias columns (q01, k01, q2, k2), partition
                        half-masks (cols 6/7), zero/one columns (8/9) used
                        as constant sources for tile padding
  yT     [4, 768, 512]  partial output, seq-block-major bf16

Layout notes:
  - Q/K land in per-head zero-padded f32r tiles (head j occupies partition
    rows 0:64 or 64:128, other half zeroed) so score matmuls run at K=128.
  - Scores are computed transposed ([k, q]) so the exp'd weights feed the
    attnV matmul directly; softmax denominators come from a ones column
    appended to V (row 64 of the attnV accumulator).  No max-subtraction:
    scores ~ N(0,1) here and fp32 exp is safe far beyond that.
  - exp'd weights (wt), V (vaug), and the mask are bf16: 2x DVE rate for
    the mask multiplies and half the SBUF traffic.
  - 1/sum is a single custom-DVE reciprocal_approx_fast (~51 ULP) off a
    partition-0-based SBUF copy (the custom op corrupts data from PSUM or
    nonzero partition bases), then a DVE cast to f32r feeds the K=1
    ones-matmul broadcast.  No Ln on ACT, so the Exp table stays resident
    the whole kernel (exactly one ACT_TABLE_LOAD).
  - QKV bias epilogues run on ACT (Identity with per-partition scale/bias)
    during the projection phase where ACT is otherwise idle.
  - v-bias and b_out are folded on the host (exact: sum_k softmax = 1).
  - DMA queues: x on sync, q/k + wout weights on gpsimd, V weights + mask
    on scalar - three parallel queues; bqkv rides first on sync to unblock
    the memset-sourced PE warmup (6 f32 matmuls, no DMA dependency) that
    lifts the HAM clock gate to 8/8 during the fill.
  - The last q-block fuses prep/normalize per head and drains over all
    three DMA queues with copies split between DVE and ACT.
"""

import os

import numpy as np

B, S, E, H, D = 2, 2048, 768, 12, 64
NCORES = 8

_CACHE = {}

# Results of the last traced run (test.py reads this)
LAST_RESULTS = None


def _build_nc(bf16io=True, diag_first=False):
    import concourse.bacc as bacc
    import concourse.mybir as mybir
    import concourse.tile as tile

    f32 = mybir.dt.float32
    f32r = mybir.dt.float32r
    AF = mybir.ActivationFunctionType

    nc = bacc.Bacc("TRN2", target_bir_lowering=False, debug=False)

    # I/O dtype for x / qkv weights / y: bf16 halves DMA bytes (the
    # projection matmuls run bf16 at the same 1 cycle/row, and bf16 drops
    # the f32r >=256-col rule so V needs no zero padding), f32r is exact.
    iod = mybir.dt.bfloat16 if bf16io else f32r
    oud = mybir.dt.bfloat16 if bf16io else f32
    vw_cols = 192 if bf16io else 256

    # x and y live in DRAM as [seq-block, E, 512] so every DMA moves one
    # fully-contiguous block.
    xT = nc.dram_tensor("xT", [4, E, 512], iod, kind="ExternalInput").ap()
    wqkvT = nc.dram_tensor("wqkvT", [E, 384], iod, kind="ExternalInput").ap()
    wvnat = nc.dram_tensor("wvnat", [E, vw_cols], iod,
                           kind="ExternalInput").ap()
    woutT = nc.dram_tensor("woutT", [256, E], f32r, kind="ExternalInput").ap()
    maskb = nc.dram_tensor("maskb", [128, 896], mybir.dt.bfloat16,
                           kind="ExternalInput").ap()
    bqkv = nc.dram_tensor("bqkv", [128, 10], f32, kind="ExternalInput").ap()
    ones1 = nc.dram_tensor("ones1", [65, 128], f32r, kind="ExternalInput").ap()
    yT = nc.dram_tensor("yT", [4, E, 512], oud, kind="ExternalOutput").ap()

    with tile.TileContext(nc) as tc:
        with (
            nc.allow_low_precision("fp32r matmul pipeline"),
            tc.tile_pool(name="const", bufs=1) as constp,
            tc.tile_pool(name="qkv", bufs=1) as qkvp,
            tc.tile_pool(name="work", bufs=4) as workp,
            tc.tile_pool(name="out", bufs=2) as outp,
            tc.tile_pool(name="dram", bufs=1, space="DRAM") as dramp,
        ):
            # ---- constants / weights (DMA order = need order) ------------
            # bqkv rides first on the sync queue (tiny; unblocks the PE
            # warmup ~4us earlier than the gpsimd queue would).  Weights
            # stream on the gpsimd queue in parallel with x on sync; the
            # mask/ones constants (not needed until attention) go last.
            bqkvt = constp.tile([128, 10], f32, name="bqkvt", tag="bqkvt")
            nc.sync.dma_start(bqkvt[:], bqkv[:])
            # tiny warmup source via memset - no DMA dependency, so the PE
            # warmup starts the moment the preamble ends.  Plain f32 (memset
            # can't write f32r): 4 cycles/row means each matmul streams 4x
            # longer, so fewer are needed for the HAM ramp.
            wsrc = constp.tile([128, 640], f32, name="wsrc", tag="wsrc")
            nc.vector.memset(wsrc[:], 0.25)
            wq = []
            for e in range(6):
                t = constp.tile([128, 384], iod, name=f"wq{e}", tag=f"wq{e}")
                nc.gpsimd.dma_start(t[:], wqkvT[e * 128:(e + 1) * 128, :])
                wq.append(t)
            wvn = []
            for e in range(6):
                t = constp.tile([128, vw_cols], iod, name=f"wv{e}",
                                tag=f"wv{e}")
                # scalar queue: in parallel with wq on gpsimd, so the V
                # pass of seq-block 0 isn't starved behind the q/k weights
                nc.scalar.dma_start(t[:], wvnat[e * 128:(e + 1) * 128, :])
                wvn.append(t)
            xt = []
            for e in range(6):
                t = constp.tile([128, S], iod, name=f"xt{e}", tag=f"xt{e}")
                xt.append(t)
            for e in range(6):
                nc.sync.dma_start(xt[e][:, 0:512],
                                  xT[0, e * 128:(e + 1) * 128, :])
            maskt = constp.tile([128, 896], mybir.dt.bfloat16, name="maskt",
                                tag="maskt")
            nc.scalar.dma_start(maskt[:], maskb[:])
            onest = constp.tile([65, 128], f32r, name="onest", tag="onest")
            nc.scalar.dma_start(onest[:], ones1[:])
            for sb in range(1, 4):
                for e in range(6):
                    nc.sync.dma_start(
                        xt[e][:, sb * 512:(sb + 1) * 512],
                        xT[sb, e * 128:(e + 1) * 128, :],
                    )
            woutp_t = constp.tile([128, E], f32r, name="woutp", tag="woutp")
            nc.gpsimd.dma_start(woutp_t[:], woutT[0:128, :])
            wout2_t = constp.tile([128, E], f32r, name="wout2", tag="wout2")
            nc.gpsimd.dma_start(wout2_t[:], woutT[128:256, :])

            # ---- PE warmup: lift HAM to 8/8 while DMAs stream ------------
            with tc.tile_pool(name="psW", bufs=1, space="PSUM") as psW:
                warm = psW.tile([128, 512], f32, name="warm", tag="warm")
                for i in range(6):
                    nc.tensor.matmul(warm[:], lhsT=wsrc[:, 0:128],
                                     rhs=wsrc[:, 128:640],
                                     start=(i == 0), stop=(i == 5))
                wsb = outp.tile([128, 512], f32, name="wsb", tag="wsb",
                                bufs=1)
                nc.vector.tensor_copy(wsb[:], warm[:])

            # ---- QKV projection into zero-padded per-head tiles ----------
            qz = []
            kz = []
            for h in range(3):
                tq = qkvp.tile([128, S], f32r, name=f"q{h}z", tag=f"q{h}z")
                tk = qkvp.tile([128, S], f32r, name=f"k{h}z", tag=f"k{h}z")
                qz.append(tq)
                kz.append(tk)
            # vaug[0]/vaug[2]: per 128-kpos chunk [V(64 cols) | ones]  ->
            #   attnV rows 0:63 = attnT, row 64 = softmax denominators.
            # vaug[1]: [ones | zeros(63) | V(64)] -> attnV row 0 = sums,
            #   rows 64:127 = attnT, so h1 lands at partition base 64 of
            #   at01 with no partition-shifting DMA.
            vaug = []
            vw = [65, 128, 65]
            for h in range(3):
                t = qkvp.tile([128, 16 * vw[h]], mybir.dt.bfloat16,
                              name=f"vaug{h}", tag=f"vaug{h}")
                vaug.append(t)
                r3 = t.rearrange("p (c w) -> p c w", w=vw[h])
                onescol = 64 if h != 1 else 0
                nc.vector.tensor_copy(
                    r3[:, :, onescol:onescol + 1],
                    bqkvt[:, 9:10].unsqueeze(1).broadcast_to([128, 16, 1]))
            nc.vector.tensor_copy(
                vaug[1].rearrange("p (c w) -> p c w", w=128)[:, :, 1:64],
                bqkvt[:, 8:9].unsqueeze(1).broadcast_to([128, 16, 63]))

            # zero the padded halves that no projection pass writes
            # (free-dim broadcast of an all-zero mask column)
            zsrc = bqkvt[64:128, 8:9].broadcast_to([64, S])
            nc.vector.tensor_copy(qz[2][64:128, :], zsrc)
            nc.vector.tensor_copy(kz[2][64:128, :], zsrc)

            # projection passes: (col offset, M, which tiles)
            passes = [(0, 128), (128, 128), (256, 64), (320, 64)]

            with tc.tile_pool(name="psA", bufs=2, space="PSUM") as psA:
                for sb in range(4):
                    ssl = slice(sb * 512, (sb + 1) * 512)
                    for ot in range(4):
                        off, M = passes[ot]
                        ps = psA.tile([M, 512], f32, name="qkvps",
                                      tag="qkvps")
                        for e in range(6):
                            nc.tensor.matmul(
                                ps[:],
                                lhsT=wq[e][:, off:off + M],
                                rhs=xt[e][:, ssl],
                                start=(e == 0),
                                stop=(e == 5),
                            )
                        # fused epilogue: out = ps*halfmask + bias via ACT
                        # Identity (bias cols pre-masked on host; cols 6/7
                        # are [1;0] / [0;1] partition masks).  ACT is idle
                        # during the projection phase and Identity is in
                        # every ACT table set, so this is free and keeps
                        # DVE clear for the attention phase.
                        mt = bqkvt[:, 6:7]
                        mb = bqkvt[:, 7:8]
                        if ot == 0:  # q0 top, q1 bottom
                            nc.scalar.activation(
                                qz[0][:, ssl], ps[:], AF.Identity,
                                bias=bqkvt[:, 0:1], scale=mt)
                            nc.scalar.activation(
                                qz[1][:, ssl], ps[:], AF.Identity,
                                bias=bqkvt[:, 1:2], scale=mb)
                        elif ot == 1:  # k0 top, k1 bottom
                            nc.scalar.activation(
                                kz[0][:, ssl], ps[:], AF.Identity,
                                bias=bqkvt[:, 2:3], scale=mt)
                            nc.scalar.activation(
                                kz[1][:, ssl], ps[:], AF.Identity,
                                bias=bqkvt[:, 3:4], scale=mb)
                        elif ot == 2:  # q2 (M=64; bottom zeroed up-front)
                            nc.scalar.activation(
                                qz[2][0:64, ssl], ps[:], AF.Identity,
                                bias=bqkvt[0:64, 4:5])
                        else:  # k2
                            nc.scalar.activation(
                                kz[2][0:64, ssl], ps[:], AF.Identity,
                                bias=bqkvt[0:64, 5:6])

                    # V natural-layout projection for this seq block
                    for st in range(sb * 4, sb * 4 + 4):
                        pv = psA.tile([128, vw_cols], f32, name="vnat",
                                      tag="vnat")
                        for e in range(6):
                            nc.tensor.matmul(
                                pv[:],
                                lhsT=xt[e][:, st * 128:(st + 1) * 128],
                                rhs=wvn[e][:],
                                start=(e == 0),
                                stop=(e == 5),
                            )
                        nc.vector.tensor_copy(
                            vaug[0][:, st * 65:st * 65 + 64], pv[:, 0:64])
                        nc.vector.tensor_copy(
                            vaug[1][:, st * 128 + 64:st * 128 + 128],
                            pv[:, 64:128])
                        nc.vector.tensor_copy(
                            vaug[2][:, st * 65:st * 65 + 64], pv[:, 128:192])

            # ---- attention + output projection ---------------------------
            at01 = qkvp.tile([128, S], f32r, name="at01", tag="at01")
            at2z = qkvp.tile([128, S], f32r, name="at2z", tag="at2z")

            # zero the padded bottom half of at2z once
            nc.vector.tensor_copy(at2z[64:128, :],
                                  bqkvt[64:128, 8:9].broadcast_to([64, S]))

            with tc.tile_pool(name="psB", bufs=1, space="PSUM") as psB:

                def attention(qb, after_first=None, descend=False):
                    nkc = 4 * (qb + 1)
                    qsl = slice(qb * 512, (qb + 1) * 512)
                    pa = []
                    for h in range(3):
                        t = psB.tile([65 if h != 1 else 128, 512], f32,
                                     name=f"pa{h}", tag="pa", bufs=3)
                        pa.append(t)

                    # kc2 group order: ascending (off-diagonal first).
                    # Measured in-process: diag-first and descending both
                    # lose ~10us - the diagonal chains compound with the
                    # boundary backlog instead of hiding in it.
                    if diag_first:
                        kc2s = ([4 * qb, 4 * qb + 2]
                                + list(range(0, 4 * qb, 2)))
                    else:
                        kc2s = list(range(0, nkc, 2))
                    if descend:
                        kc2s = list(range(0, nkc, 2))[::-1]
                    first_kc, last_kc = kc2s[0], kc2s[-1] + 1

                    def emit_attnv(h, kc2, wt):
                        for j in range(2):
                            kc = kc2 + j
                            # in descend mode diag chunks run full width
                            # (their dead cols are mask-zeroed) so the
                            # first emitted chunk's start covers all of pa
                            nw = 0 if descend else \
                                128 * max(0, kc - 4 * qb)
                            nc.tensor.matmul(
                                pa[h][:, nw:512],
                                lhsT=vaug[h].rearrange(
                                    "p (c w) -> p c w", w=vw[h])[:, kc, :],
                                rhs=wt[:, j * 512 + nw:(j + 1) * 512],
                                start=(kc == first_kc),
                                stop=(kc == last_kc),
                            )

                    # one-group software pipeline: emit group g's attnVs
                    # after group g+1's scores.  The previous q-block's
                    # broadcast+normalize (finish_a) goes FIRST: its
                    # inputs are long ready, so its matmuls fill the
                    # boundary bubble while the first score group waits
                    # for the previous block's exp backlog to drain.
                    if after_first is not None:
                        after_first[0]()
                    pend = []
                    for gi, kc2 in enumerate(kc2s):
                        cur = []
                        for h in range(3):
                            ps = psB.tile([128, 1024], f32, name="ps",
                                          tag="ps", bufs=2)
                            wt = workp.tile([128, 1024], mybir.dt.bfloat16,
                                            name="wt", tag="wt", bufs=6)
                            for j in range(2):
                                kc = kc2 + j
                                ksl = slice(kc * 128, (kc + 1) * 128)
                                # diagonal chunks: cols < 128c are fully
                                # masked - skip them in the score matmul
                                # (clamped so the moving dim stays >= 256,
                                # the f32r full-rate threshold).  descend
                                # mode computes full width instead (the
                                # mask zeroes the dead region, so the
                                # first chunk's start covers all of pa).
                                c = kc - 4 * qb
                                ms = 0 if (c <= 0 or descend) \
                                    else min(128 * c, 256)
                                nc.tensor.matmul(
                                    ps[:, j * 512 + ms:(j + 1) * 512],
                                    lhsT=kz[h][:, ksl],
                                    rhs=qz[h][:, qb * 512 + ms:
                                              (qb + 1) * 512],
                                    start=True, stop=True)
                            if kc2 - 4 * qb >= 0 and not descend:
                                # diag pair: exp only the live spans (cols
                                # < 128c of chunk c are fully masked;
                                # nothing below reads them)
                                e0 = 128 * (kc2 - 4 * qb)
                                e1 = e0 + 128
                                nc.scalar.activation(
                                    wt[:, e0:512], ps[:, e0:512], AF.Exp,
                                    scale=0.125)
                                nc.scalar.activation(
                                    wt[:, 512 + e1:1024],
                                    ps[:, 512 + e1:1024],
                                    AF.Exp, scale=0.125)
                            else:
                                nc.scalar.activation(wt[:], ps[:], AF.Exp,
                                                     scale=0.125)
                            for j in range(2):
                                kc = kc2 + j
                                if kc >= 4 * qb:
                                    c = kc - 4 * qb
                                    off = 384 - 128 * c
                                    # cols < 128c are fully masked: skip
                                    # them here and in the attnV (their
                                    # contribution is exactly zero); in
                                    # descend mode mask the full width
                                    # (cols < 384 of maskb are all-zero)
                                    m0 = 0 if descend else 128 * c
                                    nc.vector.tensor_mul(
                                        wt[:, j * 512 + m0:
                                           (j + 1) * 512],
                                        wt[:, j * 512 + m0:
                                           (j + 1) * 512],
                                        maskt[:, off + m0:off + 512])
                            cur.append((h, kc2, wt))
                        if gi == 0 and after_first is not None:
                            # splice the previous q-block's projection
                            # after the first score group: by now the
                            # normalize mults have landed
                            after_first[1]()
                            after_first = None
                        for args in pend:
                            emit_attnv(*args)
                        pend = cur
                    for args in pend:
                        emit_attnv(*args)
                    return pa

                def prep_normalize(qb, pa, fuse_finish=False):
                    # All pa readers live here (attnT copy + reciprocal of
                    # the sums row) so the accumulator banks free before
                    # the next q-block's attention needs them.  1/s is a
                    # single custom-DVE op (~51 ULP), then an ACT Copy
                    # casts it to f32r for the broadcast matmul - no ACT
                    # table switches, so Exp stays resident all kernel.
                    aus, rrs = {}, {}
                    for h in (0, 1, 2):
                        au = workp.tile([128, 512], f32, name="au",
                                        tag="au", bufs=3)
                        # copy the sums row along with the attn rows (same
                        # DVE cost - partitions run in parallel): custom
                        # DVE ops cannot read PSUM, so the reciprocal
                        # needs its input in SBUF.  Per-head fused chain in
                        # attnV completion order so the last q-block's
                        # normalize starts as soon as each head lands.
                        hsl = slice(0, 128) if h == 1 else slice(0, 65)
                        nc.vector.tensor_copy(au[hsl, :], pa[h][hsl, :])
                        aus[h] = au
                        rs = slice(0, 1) if h == 1 else slice(64, 65)
                        nrow = 65 if h != 1 else 1
                        rsb = workp.tile([65, 512], f32, name="rsb",
                                         tag="rsb", bufs=3)
                        # full-height recip: the custom-DVE op silently
                        # corrupts data when its AP starts at a nonzero
                        # partition, and extra partitions are free (the
                        # non-sums rows are garbage and never read)
                        nc.vector.reciprocal_approx_fast(
                            out=rsb[0:nrow, :], in_=au[0:nrow, :])
                        rr = workp.tile([65, 512], f32r, name="rr",
                                        tag="rr", bufs=3)
                        # f32->f32r cast on DVE: keeps the broadcast's
                        # dependency chain off the ACT queue, which still
                        # owes the current q-block's exps
                        nc.vector.tensor_copy(rr[rs, :], rsb[rs, :])
                        rrs[h] = rr
                        if fuse_finish:
                            # last q-block: broadcast + normalize per head
                            # immediately so the tail chain overlaps the
                            # remaining heads' attnV
                            qsl = slice(qb * 512, (qb + 1) * 512)
                            os_ = (onest[0:1, :] if h == 1
                                   else onest[64:65, :])
                            pbs = psB.tile([128, 512], f32, name="pbs",
                                           tag="py", bufs=1)
                            nc.tensor.matmul(pbs[:], lhsT=os_,
                                             rhs=rr[rs, :],
                                             start=True, stop=True)
                            if h == 0:
                                nc.vector.tensor_mul(
                                    at01[0:64, qsl], au[0:64, :],
                                    pbs[0:64, :])
                            elif h == 1:
                                nc.vector.tensor_mul(
                                    at01[64:128, qsl], au[64:128, :],
                                    pbs[64:128, :])
                            else:
                                nc.vector.tensor_mul(
                                    at2z[0:64, qsl], au[0:64, :],
                                    pbs[0:64, :])
                    return aus, rrs

                def finish_a(qb, aus, rrs):
                    # broadcast reciprocals (K=1 ones matmul), normalize.
                    # Emitted after the NEXT q-block's attention so the
                    # in-order PE stream never stalls on the DVE chain.
                    qsl = slice(qb * 512, (qb + 1) * 512)
                    for h in (1, 0, 2):
                        rs = slice(0, 1) if h == 1 else slice(64, 65)
                        os_ = onest[0:1, :] if h == 1 else onest[64:65, :]
                        pbs = psB.tile([128, 512], f32, name="pbs",
                                       tag="py", bufs=1)
                        nc.tensor.matmul(pbs[:], lhsT=os_,
                                         rhs=rrs[h][rs, :],
                                         start=True, stop=True)
                        if h == 1:
                            nc.vector.tensor_mul(
                                at01[64:128, qsl], aus[1][64:128, :],
                                pbs[64:128, :])
                        elif h == 0:
                            nc.vector.tensor_mul(
                                at01[0:64, qsl], aus[0][0:64, :],
                                pbs[0:64, :])
                        else:
                            nc.vector.tensor_mul(
                                at2z[0:64, qsl], aus[2][0:64, :],
                                pbs[0:64, :])
                def finish_b(qb, aus, rrs, last=False):
                    qsl = slice(qb * 512, (qb + 1) * 512)
                    for et in range(6):
                        esl = slice(et * 128, (et + 1) * 128)
                        # on the final q-block nothing contends for the
                        # score slots - alternate banks so the drain
                        # overlaps the next matmul pair
                        ptag = "ps" if (last and et % 2) else "py"
                        py = psB.tile([128, 512], f32, name="py", tag=ptag,
                                      bufs=2 if ptag == "ps" else 1)
                        nc.tensor.matmul(py[:], lhsT=woutp_t[:, esl],
                                         rhs=at01[:, qsl],
                                         start=True, stop=False)
                        nc.tensor.matmul(py[:], lhsT=wout2_t[:, esl],
                                         rhs=at2z[:, qsl],
                                         start=False, stop=True)
                        yt_t = outp.tile([128, 512], oud, name="yt",
                                         tag="yt")
                        if last:
                            # tail: split the PSUM->SBUF copies between
                            # DVE and the idle ACT, and the drain across
                            # four DMA queues
                            if et % 2 == 0:
                                nc.vector.tensor_copy(yt_t[:], py[:])
                            else:
                                nc.scalar.activation(yt_t[:], py[:],
                                                     AF.Copy)
                            eng = (nc.sync, nc.gpsimd, nc.scalar)[et % 3]
                            eng.dma_start(yT[qb, esl, :], yt_t[:])
                        else:
                            nc.vector.tensor_copy(yt_t[:], py[:])
                            nc.sync.dma_start(yT[qb, esl, :], yt_t[:])

                pending = None
                order = (0, 1, 2, 3)
                for qi, qb in enumerate(order):
                    cb = None
                    if pending:
                        cb = (lambda p=pending: finish_a(*p),
                              lambda p=pending: finish_b(*p))
                    pa = attention(qb, after_first=cb)
                    if qi == len(order) - 1:
                        aus, rrs = prep_normalize(qb, pa, fuse_finish=True)
                        finish_b(qb, aus, rrs, last=True)
                    else:
                        aus, rrs = prep_normalize(qb, pa)
                        pending = (qb, aus, rrs)

    nc.compile()
    return nc


BF16IO = bool(int(os.environ.get("KERNEL_BF16IO", "1")))


DIAG_FIRST = bool(int(os.environ.get("KERNEL_DIAG_FIRST", "0")))


def _get_nc(bf16io=None, diag_first=None):
    if bf16io is None:
        bf16io = BF16IO
    if diag_first is None:
        diag_first = DIAG_FIRST
    key = ("nc", bf16io, diag_first)
    if key not in _CACHE:
        _CACHE[key] = _build_nc(bf16io, diag_first)
    return _CACHE[key]


def _host_inputs(x, w_qkv, b_qkv, w_out, bf16io=None):
    """Build the 8 per-core input maps."""
    from ml_dtypes import bfloat16
    if bf16io is None:
        bf16io = BF16IO
    iodt = bfloat16 if bf16io else np.float32
    mbig = np.zeros((128, 896), np.float32)
    kp = np.arange(128)[:, None]
    t = np.arange(896)[None, :]
    mbig[kp <= t - 384] = 1.0
    mbig = mbig.astype(bfloat16)
    w_outT = np.ascontiguousarray(w_out.T)

    in_maps = []
    for core in range(NCORES):
        b, g = divmod(core, 4)
        base = 192 * g
        xTc = np.ascontiguousarray(
            x[b].T.reshape(E, 4, 512).transpose(1, 0, 2)).astype(iodt)
        q01 = w_qkv[base:base + 128]
        q2 = w_qkv[base + 128:base + 192]
        k01 = w_qkv[768 + base:768 + base + 128]
        k2 = w_qkv[768 + base + 128:768 + base + 192]
        wsl = np.concatenate([q01, k01, q2, k2], axis=0)       # [384, E]
        wqkvTc = np.ascontiguousarray(wsl.T).astype(iodt)
        if bf16io:
            wv = w_qkv[1536 + base:1536 + base + 192]
        else:
            wv = np.zeros((256, E), np.float32)
            wv[0:192] = w_qkv[1536 + base:1536 + base + 192]
        wvnatc = np.ascontiguousarray(wv.T).astype(iodt)
        bq = np.zeros((128, 10), np.float32)
        bq[:, 9] = 1.0
        bq[0:64, 0] = b_qkv[base:base + 64]               # q0 (top half)
        bq[64:128, 1] = b_qkv[base + 64:base + 128]       # q1 (bottom half)
        bq[0:64, 2] = b_qkv[768 + base:768 + base + 64]   # k0
        bq[64:128, 3] = b_qkv[768 + base + 64:768 + base + 128]  # k1
        bq[0:64, 4] = b_qkv[base + 128:base + 192]        # q2
        bq[0:64, 5] = b_qkv[768 + base + 128:768 + base + 192]   # k2
        bq[0:64, 6] = 1.0                                 # top-half mask
        bq[64:128, 7] = 1.0                               # bottom-half mask
        wo = np.zeros((256, E), np.float32)
        wo[0:192] = w_outT[base:base + 192]
        onesv = np.zeros((65, 128), np.float32)
        onesv[0, :] = 1.0
        onesv[64, :] = 1.0
        in_maps.append({
            "xT": xTc, "wqkvT": wqkvTc, "wvnat": wvnatc,
            "woutT": np.ascontiguousarray(wo), "maskb": mbig, "bqkv": bq,
            "ones1": onesv,
        })
    return in_maps


def _reference_numpy(x, mask, w_qkv, b_qkv, w_out, b_out):
    """Fallback for non-causal masks (never expected for this problem)."""
    b, s, _ = x.shape
    qkv = x @ w_qkv.T + b_qkv
    qkv = qkv.reshape(b, s, 3, H, D).transpose(2, 0, 3, 1, 4)
    q, k, v = qkv[0], qkv[1], qkv[2]
    scores = np.einsum("bhqd,bhkd->bhqk", q, k) * (D ** -0.5)
    scores = np.where(mask == 0, -np.inf, scores)
    scores -= scores.max(axis=-1, keepdims=True)
    w = np.exp(scores)
    w /= w.sum(axis=-1, keepdims=True)
    attn = np.einsum("bhqk,bhkd->bhqd", w, v)
    attn = attn.transpose(0, 2, 1, 3).reshape(b, s, E)
    return (attn @ w_out.T + b_out).astype(np.float32)


def kernel(x, mask, w_qkv, b_qkv, w_out, b_out):
    global LAST_RESULTS
    x = np.asarray(x, np.float32)
    mask = np.asarray(mask)
    w_qkv = np.asarray(w_qkv, np.float32)
    b_qkv = np.asarray(b_qkv, np.float32)
    w_out = np.asarray(w_out, np.float32)
    b_out = np.asarray(b_out, np.float32)

    # The device kernel bakes in causality; verify and fall back otherwise.
    m2 = np.asarray(mask).reshape(mask.shape[-2], mask.shape[-1])
    expect = np.tril(np.ones((S, S), m2.dtype))
    if m2.shape != (S, S) or not np.array_equal(m2, expect):
        return _reference_numpy(x, mask, w_qkv, b_qkv, w_out, b_out)

    from concourse.bass_utils import run_bass_kernel_spmd

    nc = _get_nc()
    in_maps = _host_inputs(x, w_qkv, b_qkv, w_out)
    trace = bool(int(os.environ.get("KERNEL_TRACE", "0")))
    kwargs = {}
    if trace:
        kwargs["trace"] = True
        kwargs["trace_cores"] = list(range(NCORES))
    res = run_bass_kernel_spmd(nc, in_maps, core_ids=list(range(NCORES)),
                               **kwargs)
    LAST_RESULTS = res

    # v-bias flows through the (normalized) attention as an additive
    # constant: y += w_out @ b_v.  Exact because softmax rows sum to 1.
    b_eff = b_out + w_out @ b_qkv[2 * E:]
    y = np.empty((B, S, E), np.float32)
    for b in range(B):
        acc = res.results[b * 4]["yT"].astype(np.float32).copy()
        for g in range(1, 4):
            acc += res.results[b * 4 + g]["yT"]
        # acc is [4, E, 512] (seq-block major) -> [S, E]
        y[b] = acc.transpose(0, 2, 1).reshape(S, E) + b_eff
    return y



# revision 77
# speedup vs baseline: 1.1693x; 1.1693x over previous
"""Multi-head attention (B=2, S=2048, E=768, H=12, D=64) on 8 trn2 NeuronCores.

Sharding: tensor-parallel over heads x data-parallel over batch.
Core c handles batch b = c // 4 and heads {3g, 3g+1, 3g+2} for g = c % 4.
Each core computes the QKV projection for its 3 heads over the full
sequence, causal attention, and a partial output projection
(w_out sliced along its input dim).  The host sums the 4 partial outputs
per batch (the unshard required by input-dim-sharded w_out).

Device inputs (per core, SPMD - same program, different data; x / qkv
weights / mask / y ship as bf16 - halves DMA bytes at the same
1 cycle/row matmul rate, ~0.4% rms error vs the 2e-2 budget):
  xT     [4, 768, 512]  x[b] transposed, seq-block-major so every DMA is
                        one fully-contiguous block (2x effective DMA rate
                        vs strided [E, S] slices)
  wqkvT  [768, 384]     q/k weight slices, transposed; column order:
                        [q_h0|q_h1 (128), k_h0|k_h1 (128), q2 (64), k2 (64)]
  wvnat  [768, 192]     v weights for the 3 heads (bf16 needs no f32r
                        256-col padding)
  woutT  [256, 768]     w_out.T rows for this core's features (f32r)
  maskb  [128, 896]     sliding causal mask: maskb[kp, t] = 1 iff kp <= t-384
  bqkv   [128, 10]      per-pass bias columns (q01, k01, q2, k2), partition
                        half-masks (cols 6/7), zero/one columns (8/9) used
                        as constant sources for tile padding
  yT     [4, 768, 512]  partial output, seq-block-major bf16

Layout notes:
  - Q/K land in per-head zero-padded f32r tiles (head j occupies partition
    rows 0:64 or 64:128, other half zeroed) so score matmuls run at K=128.
  - Scores are computed transposed ([k, q]) so the exp'd weights feed the
    attnV matmul directly; softmax denominators come from a ones column
    appended to V (row 64 of the attnV accumulator).  No max-subtraction:
    scores ~ N(0,1) here and fp32 exp is safe far beyond that.
  - exp'd weights (wt), V (vaug), and the mask are bf16: 2x DVE rate for
    the mask multiplies and half the SBUF traffic.
  - 1/sum is a single custom-DVE reciprocal_approx_fast (~51 ULP) off a
    partition-0-based SBUF copy (the custom op corrupts data from PSUM or
    nonzero partition bases), then a DVE cast to f32r feeds the K=1
    ones-matmul broadcast.  No Ln on ACT, so the Exp table stays resident
    the whole kernel (exactly one ACT_TABLE_LOAD).
  - QKV bias epilogues run on ACT (Identity with per-partition scale/bias)
    during the projection phase where ACT is otherwise idle.
  - v-bias and b_out are folded on the host (exact: sum_k softmax = 1).
  - DMA queues: x on sync, q/k + wout weights on gpsimd, V weights + mask
    on scalar - three parallel queues; bqkv rides first on sync to unblock
    the memset-sourced PE warmup (6 f32 matmuls, no DMA dependency) that
    lifts the HAM clock gate to 8/8 during the fill.
  - The last q-block fuses prep/normalize per head and drains over all
    three DMA queues with copies split between DVE and ACT.
"""

import os

import numpy as np

B, S, E, H, D = 2, 2048, 768, 12, 64
NCORES = 8

_CACHE = {}

# Results of the last traced run (test.py reads this)
LAST_RESULTS = None


def _build_nc(bf16io=True, diag_first=False):
    import concourse.bacc as bacc
    import concourse.mybir as mybir
    import concourse.tile as tile

    f32 = mybir.dt.float32
    f32r = mybir.dt.float32r
    AF = mybir.ActivationFunctionType

    nc = bacc.Bacc("TRN2", target_bir_lowering=False, debug=False)

    # I/O dtype for x / qkv weights / y: bf16 halves DMA bytes (the
    # projection matmuls run bf16 at the same 1 cycle/row, and bf16 drops
    # the f32r >=256-col rule so V needs no zero padding), f32r is exact.
    iod = mybir.dt.bfloat16 if bf16io else f32r
    oud = mybir.dt.bfloat16 if bf16io else f32
    vw_cols = 192 if bf16io else 256

    # x and y live in DRAM as [seq-block, E, 512] so every DMA moves one
    # fully-contiguous block.
    xT = nc.dram_tensor("xT", [4, E, 512], iod, kind="ExternalInput").ap()
    wqkvT = nc.dram_tensor("wqkvT", [E, 384], iod, kind="ExternalInput").ap()
    wvnat = nc.dram_tensor("wvnat", [E, vw_cols], iod,
                           kind="ExternalInput").ap()
    woutT = nc.dram_tensor("woutT", [256, E], f32r, kind="ExternalInput").ap()
    maskb = nc.dram_tensor("maskb", [128, 896], mybir.dt.bfloat16,
                           kind="ExternalInput").ap()
    bqkv = nc.dram_tensor("bqkv", [128, 10], f32, kind="ExternalInput").ap()
    ones1 = nc.dram_tensor("ones1", [65, 128], f32r, kind="ExternalInput").ap()
    yT = nc.dram_tensor("yT", [4, E, 512], oud, kind="ExternalOutput").ap()

    with tile.TileContext(nc) as tc:
        with (
            nc.allow_low_precision("fp32r matmul pipeline"),
            tc.tile_pool(name="const", bufs=1) as constp,
            tc.tile_pool(name="qkv", bufs=1) as qkvp,
            tc.tile_pool(name="work", bufs=4) as workp,
            tc.tile_pool(name="out", bufs=2) as outp,
            tc.tile_pool(name="dram", bufs=1, space="DRAM") as dramp,
        ):
            # ---- constants / weights (DMA order = need order) ------------
            # bqkv rides first on the sync queue (tiny; unblocks the PE
            # warmup ~4us earlier than the gpsimd queue would).  Weights
            # stream on the gpsimd queue in parallel with x on sync; the
            # mask/ones constants (not needed until attention) go last.
            bqkvt = constp.tile([128, 10], f32, name="bqkvt", tag="bqkvt")
            nc.sync.dma_start(bqkvt[:], bqkv[:])
            # tiny warmup source via memset - no DMA dependency, so the PE
            # warmup starts the moment the preamble ends.  Plain f32 (memset
            # can't write f32r): 4 cycles/row means each matmul streams 4x
            # longer, so fewer are needed for the HAM ramp.
            wsrc = constp.tile([128, 640], f32, name="wsrc", tag="wsrc")
            nc.vector.memset(wsrc[:], 0.25)
            wq = []
            for e in range(6):
                t = constp.tile([128, 384], iod, name=f"wq{e}", tag=f"wq{e}")
                nc.gpsimd.dma_start(t[:], wqkvT[e * 128:(e + 1) * 128, :])
                wq.append(t)
            wvn = []
            for e in range(6):
                t = constp.tile([128, vw_cols], iod, name=f"wv{e}",
                                tag=f"wv{e}")
                # scalar queue: in parallel with wq on gpsimd, so the V
                # pass of seq-block 0 isn't starved behind the q/k weights
                nc.scalar.dma_start(t[:], wvnat[e * 128:(e + 1) * 128, :])
                wvn.append(t)
            xt = []
            for e in range(6):
                t = constp.tile([128, S], iod, name=f"xt{e}", tag=f"xt{e}")
                xt.append(t)
            for e in range(6):
                nc.sync.dma_start(xt[e][:, 0:512],
                                  xT[0, e * 128:(e + 1) * 128, :])
            maskt = constp.tile([128, 896], mybir.dt.bfloat16, name="maskt",
                                tag="maskt")
            nc.scalar.dma_start(maskt[:], maskb[:])
            onest = constp.tile([65, 128], f32r, name="onest", tag="onest")
            nc.scalar.dma_start(onest[:], ones1[:])
            for sb in range(1, 4):
                for e in range(6):
                    nc.sync.dma_start(
                        xt[e][:, sb * 512:(sb + 1) * 512],
                        xT[sb, e * 128:(e + 1) * 128, :],
                    )
            woutp_t = constp.tile([128, E], f32r, name="woutp", tag="woutp")
            nc.gpsimd.dma_start(woutp_t[:], woutT[0:128, :])
            wout2_t = constp.tile([128, E], f32r, name="wout2", tag="wout2")
            nc.gpsimd.dma_start(wout2_t[:], woutT[128:256, :])

            # ---- PE warmup: lift HAM to 8/8 while DMAs stream ------------
            with tc.tile_pool(name="psW", bufs=1, space="PSUM") as psW:
                warm = psW.tile([128, 512], f32, name="warm", tag="warm")
                for i in range(6):
                    nc.tensor.matmul(warm[:], lhsT=wsrc[:, 0:128],
                                     rhs=wsrc[:, 128:640],
                                     start=(i == 0), stop=(i == 5))
                wsb = outp.tile([128, 512], f32, name="wsb", tag="wsb",
                                bufs=1)
                nc.vector.tensor_copy(wsb[:], warm[:])

            # ---- QKV projection into zero-padded per-head tiles ----------
            qz = []
            kz = []
            for h in range(3):
                tq = qkvp.tile([128, S], f32r, name=f"q{h}z", tag=f"q{h}z")
                tk = qkvp.tile([128, S], f32r, name=f"k{h}z", tag=f"k{h}z")
                qz.append(tq)
                kz.append(tk)
            # vaug[0]/vaug[2]: per 128-kpos chunk [V(64 cols) | ones]  ->
            #   attnV rows 0:63 = attnT, row 64 = softmax denominators.
            # vaug[1]: [ones | zeros(63) | V(64)] -> attnV row 0 = sums,
            #   rows 64:127 = attnT, so h1 lands at partition base 64 of
            #   at01 with no partition-shifting DMA.
            vaug = []
            vw = [65, 128, 65]
            for h in range(3):
                t = qkvp.tile([128, 16 * vw[h]], mybir.dt.bfloat16,
                              name=f"vaug{h}", tag=f"vaug{h}")
                vaug.append(t)
                r3 = t.rearrange("p (c w) -> p c w", w=vw[h])
                onescol = 64 if h != 1 else 0
                nc.vector.tensor_copy(
                    r3[:, :, onescol:onescol + 1],
                    bqkvt[:, 9:10].unsqueeze(1).broadcast_to([128, 16, 1]))
            nc.vector.tensor_copy(
                vaug[1].rearrange("p (c w) -> p c w", w=128)[:, :, 1:64],
                bqkvt[:, 8:9].unsqueeze(1).broadcast_to([128, 16, 63]))

            # zero the padded halves that no projection pass writes
            # (free-dim broadcast of an all-zero mask column)
            zsrc = bqkvt[64:128, 8:9].broadcast_to([64, S])
            nc.vector.tensor_copy(qz[2][64:128, :], zsrc)
            nc.vector.tensor_copy(kz[2][64:128, :], zsrc)

            # projection passes: (col offset, M, which tiles)
            passes = [(0, 128), (128, 128), (256, 64), (320, 64)]

            with tc.tile_pool(name="psA", bufs=2, space="PSUM") as psA:
                for sb in range(4):
                    ssl = slice(sb * 512, (sb + 1) * 512)
                    for ot in range(4):
                        off, M = passes[ot]
                        ps = psA.tile([M, 512], f32, name="qkvps",
                                      tag="qkvps")
                        for e in range(6):
                            nc.tensor.matmul(
                                ps[:],
                                lhsT=wq[e][:, off:off + M],
                                rhs=xt[e][:, ssl],
                                start=(e == 0),
                                stop=(e == 5),
                            )
                        # fused epilogue: out = ps*halfmask + bias via ACT
                        # Identity (bias cols pre-masked on host; cols 6/7
                        # are [1;0] / [0;1] partition masks).  ACT is idle
                        # during the projection phase and Identity is in
                        # every ACT table set, so this is free and keeps
                        # DVE clear for the attention phase.
                        mt = bqkvt[:, 6:7]
                        mb = bqkvt[:, 7:8]
                        if ot == 0:  # q0 top, q1 bottom
                            nc.scalar.activation(
                                qz[0][:, ssl], ps[:], AF.Identity,
                                bias=bqkvt[:, 0:1], scale=mt)
                            nc.scalar.activation(
                                qz[1][:, ssl], ps[:], AF.Identity,
                                bias=bqkvt[:, 1:2], scale=mb)
                        elif ot == 1:  # k0 top, k1 bottom
                            nc.scalar.activation(
                                kz[0][:, ssl], ps[:], AF.Identity,
                                bias=bqkvt[:, 2:3], scale=mt)
                            nc.scalar.activation(
                                kz[1][:, ssl], ps[:], AF.Identity,
                                bias=bqkvt[:, 3:4], scale=mb)
                        elif ot == 2:  # q2 (M=64; bottom zeroed up-front)
                            nc.scalar.activation(
                                qz[2][0:64, ssl], ps[:], AF.Identity,
                                bias=bqkvt[0:64, 4:5])
                        else:  # k2
                            nc.scalar.activation(
                                kz[2][0:64, ssl], ps[:], AF.Identity,
                                bias=bqkvt[0:64, 5:6])

                    # V natural-layout projection for this seq block
                    for st in range(sb * 4, sb * 4 + 4):
                        pv = psA.tile([128, vw_cols], f32, name="vnat",
                                      tag="vnat")
                        for e in range(6):
                            nc.tensor.matmul(
                                pv[:],
                                lhsT=xt[e][:, st * 128:(st + 1) * 128],
                                rhs=wvn[e][:],
                                start=(e == 0),
                                stop=(e == 5),
                            )
                        nc.vector.tensor_copy(
                            vaug[0][:, st * 65:st * 65 + 64], pv[:, 0:64])
                        nc.vector.tensor_copy(
                            vaug[1][:, st * 128 + 64:st * 128 + 128],
                            pv[:, 64:128])
                        nc.vector.tensor_copy(
                            vaug[2][:, st * 65:st * 65 + 64], pv[:, 128:192])

            # ---- attention + output projection ---------------------------
            at01 = qkvp.tile([128, S], f32r, name="at01", tag="at01")
            at2z = qkvp.tile([128, S], f32r, name="at2z", tag="at2z")

            # zero the padded bottom half of at2z once
            nc.vector.tensor_copy(at2z[64:128, :],
                                  bqkvt[64:128, 8:9].broadcast_to([64, S]))

            with tc.tile_pool(name="psB", bufs=1, space="PSUM") as psB:

                def attention(qb, after_first=None, descend=False):
                    nkc = 4 * (qb + 1)
                    qsl = slice(qb * 512, (qb + 1) * 512)
                    pa = []
                    for h in range(3):
                        t = psB.tile([65 if h != 1 else 128, 512], f32,
                                     name=f"pa{h}", tag="pa", bufs=3)
                        pa.append(t)

                    # kc2 group order: ascending (off-diagonal first).
                    # Measured in-process: diag-first and descending both
                    # lose ~10us - the diagonal chains compound with the
                    # boundary backlog instead of hiding in it.
                    if diag_first:
                        kc2s = ([4 * qb, 4 * qb + 2]
                                + list(range(0, 4 * qb, 2)))
                    else:
                        kc2s = list(range(0, nkc, 2))
                    if descend:
                        kc2s = list(range(0, nkc, 2))[::-1]
                    first_kc, last_kc = kc2s[0], kc2s[-1] + 1

                    def emit_attnv(h, kc2, wt):
                        for j in range(2):
                            kc = kc2 + j
                            # in descend mode diag chunks run full width
                            # (their dead cols are mask-zeroed) so the
                            # first emitted chunk's start covers all of pa
                            nw = 0 if descend else \
                                128 * max(0, kc - 4 * qb)
                            nc.tensor.matmul(
                                pa[h][:, nw:512],
                                lhsT=vaug[h].rearrange(
                                    "p (c w) -> p c w", w=vw[h])[:, kc, :],
                                rhs=wt[:, j * 512 + nw:(j + 1) * 512],
                                start=(kc == first_kc),
                                stop=(kc == last_kc),
                            )

                    # one-group software pipeline: emit group g's attnVs
                    # after group g+1's scores.  The previous q-block's
                    # broadcast+normalize (finish_a) goes FIRST: its
                    # inputs are long ready, so its matmuls fill the
                    # boundary bubble while the first score group waits
                    # for the previous block's exp backlog to drain.
                    if after_first is not None:
                        after_first[0]()
                    pend = []
                    for gi, kc2 in enumerate(kc2s):
                        cur = []
                        for h in range(3):
                            ps = psB.tile([128, 1024], f32, name="ps",
                                          tag="ps", bufs=2)
                            wt = workp.tile([128, 1024], mybir.dt.bfloat16,
                                            name="wt", tag="wt", bufs=9)
                            for j in range(2):
                                kc = kc2 + j
                                ksl = slice(kc * 128, (kc + 1) * 128)
                                # diagonal chunks: cols < 128c are fully
                                # masked - skip them in the score matmul
                                # (clamped so the moving dim stays >= 256,
                                # the f32r full-rate threshold).  descend
                                # mode computes full width instead (the
                                # mask zeroes the dead region, so the
                                # first chunk's start covers all of pa).
                                c = kc - 4 * qb
                                ms = 0 if (c <= 0 or descend) \
                                    else min(128 * c, 256)
                                nc.tensor.matmul(
                                    ps[:, j * 512 + ms:(j + 1) * 512],
                                    lhsT=kz[h][:, ksl],
                                    rhs=qz[h][:, qb * 512 + ms:
                                              (qb + 1) * 512],
                                    start=True, stop=True)
                            if kc2 - 4 * qb >= 0 and not descend:
                                # diag pair: exp only the live spans (cols
                                # < 128c of chunk c are fully masked;
                                # nothing below reads them)
                                e0 = 128 * (kc2 - 4 * qb)
                                e1 = e0 + 128
                                nc.scalar.activation(
                                    wt[:, e0:512], ps[:, e0:512], AF.Exp,
                                    scale=0.125)
                                nc.scalar.activation(
                                    wt[:, 512 + e1:1024],
                                    ps[:, 512 + e1:1024],
                                    AF.Exp, scale=0.125)
                            else:
                                nc.scalar.activation(wt[:], ps[:], AF.Exp,
                                                     scale=0.125)
                            for j in range(2):
                                kc = kc2 + j
                                if kc >= 4 * qb:
                                    c = kc - 4 * qb
                                    off = 384 - 128 * c
                                    # cols < 128c are fully masked: skip
                                    # them here and in the attnV (their
                                    # contribution is exactly zero); in
                                    # descend mode mask the full width
                                    # (cols < 384 of maskb are all-zero)
                                    m0 = 0 if descend else 128 * c
                                    nc.vector.tensor_mul(
                                        wt[:, j * 512 + m0:
                                           (j + 1) * 512],
                                        wt[:, j * 512 + m0:
                                           (j + 1) * 512],
                                        maskt[:, off + m0:off + 512])
                            cur.append((h, kc2, wt))
                        if gi == 0 and after_first is not None:
                            # splice the previous q-block's projection
                            # after the first score group: by now the
                            # normalize mults have landed
                            after_first[1]()
                            after_first = None
                        # two-group pipeline: each group's exps get a full
                        # extra group of slack before their attnVs issue,
                        # absorbing ACT jitter (the measured tail gaps
                        # were attnVs waiting on their own group's exp)
                        if len(pend) >= 2:
                            for args in pend.pop(0):
                                emit_attnv(*args)
                        pend.append(cur)
                    for grp in pend:
                        for args in grp:
                            emit_attnv(*args)
                    return pa

                def prep_normalize(qb, pa, fuse_finish=False):
                    # All pa readers live here (attnT copy + reciprocal of
                    # the sums row) so the accumulator banks free before
                    # the next q-block's attention needs them.  1/s is a
                    # single custom-DVE op (~51 ULP), then an ACT Copy
                    # casts it to f32r for the broadcast matmul - no ACT
                    # table switches, so Exp stays resident all kernel.
                    aus, rrs = {}, {}
                    for h in (0, 1, 2):
                        au = workp.tile([128, 512], f32, name="au",
                                        tag="au", bufs=3)
                        # copy the sums row along with the attn rows (same
                        # DVE cost - partitions run in parallel): custom
                        # DVE ops cannot read PSUM, so the reciprocal
                        # needs its input in SBUF.  Per-head fused chain in
                        # attnV completion order so the last q-block's
                        # normalize starts as soon as each head lands.
                        hsl = slice(0, 128) if h == 1 else slice(0, 65)
                        nc.vector.tensor_copy(au[hsl, :], pa[h][hsl, :])
                        aus[h] = au
                        rs = slice(0, 1) if h == 1 else slice(64, 65)
                        nrow = 65 if h != 1 else 1
                        rsb = workp.tile([65, 512], f32, name="rsb",
                                         tag="rsb", bufs=3)
                        # full-height recip: the custom-DVE op silently
                        # corrupts data when its AP starts at a nonzero
                        # partition, and extra partitions are free (the
                        # non-sums rows are garbage and never read)
                        nc.vector.reciprocal_approx_fast(
                            out=rsb[0:nrow, :], in_=au[0:nrow, :])
                        rr = workp.tile([65, 512], f32r, name="rr",
                                        tag="rr", bufs=3)
                        # f32->f32r cast on DVE: keeps the broadcast's
                        # dependency chain off the ACT queue, which still
                        # owes the current q-block's exps
                        nc.vector.tensor_copy(rr[rs, :], rsb[rs, :])
                        rrs[h] = rr
                        if fuse_finish:
                            # last q-block: broadcast + normalize per head
                            # immediately so the tail chain overlaps the
                            # remaining heads' attnV
                            qsl = slice(qb * 512, (qb + 1) * 512)
                            os_ = (onest[0:1, :] if h == 1
                                   else onest[64:65, :])
                            pbs = psB.tile([128, 512], f32, name="pbs",
                                           tag="py", bufs=1)
                            nc.tensor.matmul(pbs[:], lhsT=os_,
                                             rhs=rr[rs, :],
                                             start=True, stop=True)
                            if h == 0:
                                nc.vector.tensor_mul(
                                    at01[0:64, qsl], au[0:64, :],
                                    pbs[0:64, :])
                            elif h == 1:
                                nc.vector.tensor_mul(
                                    at01[64:128, qsl], au[64:128, :],
                                    pbs[64:128, :])
                            else:
                                nc.vector.tensor_mul(
                                    at2z[0:64, qsl], au[0:64, :],
                                    pbs[0:64, :])
                    return aus, rrs

                def finish_a(qb, aus, rrs):
                    # broadcast reciprocals (K=1 ones matmul), normalize.
                    # Emitted after the NEXT q-block's attention so the
                    # in-order PE stream never stalls on the DVE chain.
                    qsl = slice(qb * 512, (qb + 1) * 512)
                    for h in (1, 0, 2):
                        rs = slice(0, 1) if h == 1 else slice(64, 65)
                        os_ = onest[0:1, :] if h == 1 else onest[64:65, :]
                        pbs = psB.tile([128, 512], f32, name="pbs",
                                       tag="py", bufs=1)
                        nc.tensor.matmul(pbs[:], lhsT=os_,
                                         rhs=rrs[h][rs, :],
                                         start=True, stop=True)
                        if h == 1:
                            nc.vector.tensor_mul(
                                at01[64:128, qsl], aus[1][64:128, :],
                                pbs[64:128, :])
                        elif h == 0:
                            nc.vector.tensor_mul(
                                at01[0:64, qsl], aus[0][0:64, :],
                                pbs[0:64, :])
                        else:
                            nc.vector.tensor_mul(
                                at2z[0:64, qsl], aus[2][0:64, :],
                                pbs[0:64, :])
                def finish_b(qb, aus, rrs, last=False):
                    qsl = slice(qb * 512, (qb + 1) * 512)
                    for et in range(6):
                        esl = slice(et * 128, (et + 1) * 128)
                        # on the final q-block nothing contends for the
                        # score slots - alternate banks so the drain
                        # overlaps the next matmul pair
                        ptag = "ps" if (last and et % 2) else "py"
                        py = psB.tile([128, 512], f32, name="py", tag=ptag,
                                      bufs=2 if ptag == "ps" else 1)
                        nc.tensor.matmul(py[:], lhsT=woutp_t[:, esl],
                                         rhs=at01[:, qsl],
                                         start=True, stop=False)
                        nc.tensor.matmul(py[:], lhsT=wout2_t[:, esl],
                                         rhs=at2z[:, qsl],
                                         start=False, stop=True)
                        yt_t = outp.tile([128, 512], oud, name="yt",
                                         tag="yt")
                        if last:
                            # tail: split the PSUM->SBUF copies between
                            # DVE and the idle ACT, and the drain across
                            # four DMA queues
                            if et % 2 == 0:
                                nc.vector.tensor_copy(yt_t[:], py[:])
                            else:
                                nc.scalar.activation(yt_t[:], py[:],
                                                     AF.Copy)
                            eng = (nc.sync, nc.gpsimd, nc.scalar)[et % 3]
                            eng.dma_start(yT[qb, esl, :], yt_t[:])
                        else:
                            nc.vector.tensor_copy(yt_t[:], py[:])
                            nc.sync.dma_start(yT[qb, esl, :], yt_t[:])

                pending = None
                order = (0, 1, 2, 3)
                for qi, qb in enumerate(order):
                    cb = None
                    if pending:
                        cb = (lambda p=pending: finish_a(*p),
                              lambda p=pending: finish_b(*p))
                    pa = attention(qb, after_first=cb)
                    if qi == len(order) - 1:
                        aus, rrs = prep_normalize(qb, pa, fuse_finish=True)
                        finish_b(qb, aus, rrs, last=True)
                    else:
                        aus, rrs = prep_normalize(qb, pa)
                        pending = (qb, aus, rrs)

    nc.compile()
    return nc


BF16IO = bool(int(os.environ.get("KERNEL_BF16IO", "1")))


DIAG_FIRST = bool(int(os.environ.get("KERNEL_DIAG_FIRST", "0")))


def _get_nc(bf16io=None, diag_first=None):
    if bf16io is None:
        bf16io = BF16IO
    if diag_first is None:
        diag_first = DIAG_FIRST
    key = ("nc", bf16io, diag_first)
    if key not in _CACHE:
        _CACHE[key] = _build_nc(bf16io, diag_first)
    return _CACHE[key]


def _host_inputs(x, w_qkv, b_qkv, w_out, bf16io=None):
    """Build the 8 per-core input maps."""
    from ml_dtypes import bfloat16
    if bf16io is None:
        bf16io = BF16IO
    iodt = bfloat16 if bf16io else np.float32
    mbig = np.zeros((128, 896), np.float32)
    kp = np.arange(128)[:, None]
    t = np.arange(896)[None, :]
    mbig[kp <= t - 384] = 1.0
    mbig = mbig.astype(bfloat16)
    w_outT = np.ascontiguousarray(w_out.T)

    in_maps = []
    for core in range(NCORES):
        b, g = divmod(core, 4)
        base = 192 * g
        xTc = np.ascontiguousarray(
            x[b].T.reshape(E, 4, 512).transpose(1, 0, 2)).astype(iodt)
        q01 = w_qkv[base:base + 128]
        q2 = w_qkv[base + 128:base + 192]
        k01 = w_qkv[768 + base:768 + base + 128]
        k2 = w_qkv[768 + base + 128:768 + base + 192]
        wsl = np.concatenate([q01, k01, q2, k2], axis=0)       # [384, E]
        wqkvTc = np.ascontiguousarray(wsl.T).astype(iodt)
        if bf16io:
            wv = w_qkv[1536 + base:1536 + base + 192]
        else:
            wv = np.zeros((256, E), np.float32)
            wv[0:192] = w_qkv[1536 + base:1536 + base + 192]
        wvnatc = np.ascontiguousarray(wv.T).astype(iodt)
        bq = np.zeros((128, 10), np.float32)
        bq[:, 9] = 1.0
        bq[0:64, 0] = b_qkv[base:base + 64]               # q0 (top half)
        bq[64:128, 1] = b_qkv[base + 64:base + 128]       # q1 (bottom half)
        bq[0:64, 2] = b_qkv[768 + base:768 + base + 64]   # k0
        bq[64:128, 3] = b_qkv[768 + base + 64:768 + base + 128]  # k1
        bq[0:64, 4] = b_qkv[base + 128:base + 192]        # q2
        bq[0:64, 5] = b_qkv[768 + base + 128:768 + base + 192]   # k2
        bq[0:64, 6] = 1.0                                 # top-half mask
        bq[64:128, 7] = 1.0                               # bottom-half mask
        wo = np.zeros((256, E), np.float32)
        wo[0:192] = w_outT[base:base + 192]
        onesv = np.zeros((65, 128), np.float32)
        onesv[0, :] = 1.0
        onesv[64, :] = 1.0
        in_maps.append({
            "xT": xTc, "wqkvT": wqkvTc, "wvnat": wvnatc,
            "woutT": np.ascontiguousarray(wo), "maskb": mbig, "bqkv": bq,
            "ones1": onesv,
        })
    return in_maps


def _reference_numpy(x, mask, w_qkv, b_qkv, w_out, b_out):
    """Fallback for non-causal masks (never expected for this problem)."""
    b, s, _ = x.shape
    qkv = x @ w_qkv.T + b_qkv
    qkv = qkv.reshape(b, s, 3, H, D).transpose(2, 0, 3, 1, 4)
    q, k, v = qkv[0], qkv[1], qkv[2]
    scores = np.einsum("bhqd,bhkd->bhqk", q, k) * (D ** -0.5)
    scores = np.where(mask == 0, -np.inf, scores)
    scores -= scores.max(axis=-1, keepdims=True)
    w = np.exp(scores)
    w /= w.sum(axis=-1, keepdims=True)
    attn = np.einsum("bhqk,bhkd->bhqd", w, v)
    attn = attn.transpose(0, 2, 1, 3).reshape(b, s, E)
    return (attn @ w_out.T + b_out).astype(np.float32)


def kernel(x, mask, w_qkv, b_qkv, w_out, b_out):
    global LAST_RESULTS
    x = np.asarray(x, np.float32)
    mask = np.asarray(mask)
    w_qkv = np.asarray(w_qkv, np.float32)
    b_qkv = np.asarray(b_qkv, np.float32)
    w_out = np.asarray(w_out, np.float32)
    b_out = np.asarray(b_out, np.float32)

    # The device kernel bakes in causality; verify and fall back otherwise.
    m2 = np.asarray(mask).reshape(mask.shape[-2], mask.shape[-1])
    expect = np.tril(np.ones((S, S), m2.dtype))
    if m2.shape != (S, S) or not np.array_equal(m2, expect):
        return _reference_numpy(x, mask, w_qkv, b_qkv, w_out, b_out)

    from concourse.bass_utils import run_bass_kernel_spmd

    nc = _get_nc()
    in_maps = _host_inputs(x, w_qkv, b_qkv, w_out)
    trace = bool(int(os.environ.get("KERNEL_TRACE", "0")))
    kwargs = {}
    if trace:
        kwargs["trace"] = True
        kwargs["trace_cores"] = list(range(NCORES))
    res = run_bass_kernel_spmd(nc, in_maps, core_ids=list(range(NCORES)),
                               **kwargs)
    LAST_RESULTS = res

    # v-bias flows through the (normalized) attention as an additive
    # constant: y += w_out @ b_v.  Exact because softmax rows sum to 1.
    b_eff = b_out + w_out @ b_qkv[2 * E:]
    y = np.empty((B, S, E), np.float32)
    for b in range(B):
        acc = res.results[b * 4]["yT"].astype(np.float32).copy()
        for g in range(1, 4):
            acc += res.results[b * 4 + g]["yT"]
        # acc is [4, E, 512] (seq-block major) -> [S, E]
        y[b] = acc.transpose(0, 2, 1).reshape(S, E) + b_eff
    return y



# revision 78
# speedup vs baseline: 1.1891x; 1.0169x over previous
"""Multi-head attention (B=2, S=2048, E=768, H=12, D=64) on 8 trn2 NeuronCores.

Sharding: tensor-parallel over heads x data-parallel over batch.
Core c handles batch b = c // 4 and heads {3g, 3g+1, 3g+2} for g = c % 4.
Each core computes the QKV projection for its 3 heads over the full
sequence, causal attention, and a partial output projection
(w_out sliced along its input dim).  The host sums the 4 partial outputs
per batch (the unshard required by input-dim-sharded w_out).

Device inputs (per core, SPMD - same program, different data; x / qkv
weights / mask / y ship as bf16 - halves DMA bytes at the same
1 cycle/row matmul rate, ~0.4% rms error vs the 2e-2 budget):
  xT     [4, 768, 512]  x[b] transposed, seq-block-major so every DMA is
                        one fully-contiguous block (2x effective DMA rate
                        vs strided [E, S] slices)
  wqkvT  [768, 384]     q/k weight slices, transposed; column order:
                        [q_h0|q_h1 (128), k_h0|k_h1 (128), q2 (64), k2 (64)]
  wvnat  [768, 192]     v weights for the 3 heads (bf16 needs no f32r
                        256-col padding)
  woutT  [256, 768]     w_out.T rows for this core's features (f32r)
  maskb  [128, 896]     sliding causal mask: maskb[kp, t] = 1 iff kp <= t-384
  bqkv   [128, 10]      per-pass bias columns (q01, k01, q2, k2), partition
                        half-masks (cols 6/7), zero/one columns (8/9) used
                        as constant sources for tile padding
  yT     [4, 768, 512]  partial output, seq-block-major bf16

Layout notes:
  - Q/K land in per-head zero-padded f32r tiles (head j occupies partition
    rows 0:64 or 64:128, other half zeroed) so score matmuls run at K=128.
  - Scores are computed transposed ([k, q]) so the exp'd weights feed the
    attnV matmul directly; softmax denominators come from a ones column
    appended to V (row 64 of the attnV accumulator).  No max-subtraction:
    scores ~ N(0,1) here and fp32 exp is safe far beyond that.
  - exp'd weights (wt), V (vaug), and the mask are bf16: 2x DVE rate for
    the mask multiplies and half the SBUF traffic.
  - 1/sum is a single custom-DVE reciprocal_approx_fast (~51 ULP) off a
    partition-0-based SBUF copy (the custom op corrupts data from PSUM or
    nonzero partition bases), then a DVE cast to f32r feeds the K=1
    ones-matmul broadcast.  No Ln on ACT, so the Exp table stays resident
    the whole kernel (exactly one ACT_TABLE_LOAD).
  - QKV bias epilogues run on ACT (Identity with per-partition scale/bias)
    during the projection phase where ACT is otherwise idle.
  - v-bias and b_out are folded on the host (exact: sum_k softmax = 1).
  - DMA queues: x on sync, q/k + wout weights on gpsimd, V weights + mask
    on scalar - three parallel queues; bqkv rides first on sync to unblock
    the memset-sourced PE warmup (6 f32 matmuls, no DMA dependency) that
    lifts the HAM clock gate to 8/8 during the fill.
  - The last q-block fuses prep/normalize per head and drains over all
    three DMA queues with copies split between DVE and ACT.
"""

import os

import numpy as np

B, S, E, H, D = 2, 2048, 768, 12, 64
NCORES = 8

_CACHE = {}

# Results of the last traced run (test.py reads this)
LAST_RESULTS = None


def _build_nc(bf16io=True, diag_first=False):
    import concourse.bacc as bacc
    import concourse.mybir as mybir
    import concourse.tile as tile

    f32 = mybir.dt.float32
    f32r = mybir.dt.float32r
    AF = mybir.ActivationFunctionType

    nc = bacc.Bacc("TRN2", target_bir_lowering=False, debug=False)

    # I/O dtype for x / qkv weights / y: bf16 halves DMA bytes (the
    # projection matmuls run bf16 at the same 1 cycle/row, and bf16 drops
    # the f32r >=256-col rule so V needs no zero padding), f32r is exact.
    iod = mybir.dt.bfloat16 if bf16io else f32r
    oud = mybir.dt.bfloat16 if bf16io else f32
    vw_cols = 192 if bf16io else 256

    # x and y live in DRAM as [seq-block, E, 512] so every DMA moves one
    # fully-contiguous block.
    xT = nc.dram_tensor("xT", [4, E, 512], iod, kind="ExternalInput").ap()
    wqkvT = nc.dram_tensor("wqkvT", [E, 384], iod, kind="ExternalInput").ap()
    wvnat = nc.dram_tensor("wvnat", [E, vw_cols], iod,
                           kind="ExternalInput").ap()
    woutT = nc.dram_tensor("woutT", [256, E], f32r, kind="ExternalInput").ap()
    maskb = nc.dram_tensor("maskb", [128, 896], mybir.dt.bfloat16,
                           kind="ExternalInput").ap()
    bqkv = nc.dram_tensor("bqkv", [128, 10], f32, kind="ExternalInput").ap()
    ones1 = nc.dram_tensor("ones1", [65, 128], f32r, kind="ExternalInput").ap()
    yT = nc.dram_tensor("yT", [4, E, 512], oud, kind="ExternalOutput").ap()

    with tile.TileContext(nc) as tc:
        with (
            nc.allow_low_precision("fp32r matmul pipeline"),
            tc.tile_pool(name="const", bufs=1) as constp,
            tc.tile_pool(name="qkv", bufs=1) as qkvp,
            tc.tile_pool(name="work", bufs=4) as workp,
            tc.tile_pool(name="out", bufs=2) as outp,
            tc.tile_pool(name="dram", bufs=1, space="DRAM") as dramp,
        ):
            # ---- constants / weights (DMA order = need order) ------------
            # bqkv rides first on the sync queue (tiny; unblocks the PE
            # warmup ~4us earlier than the gpsimd queue would).  Weights
            # stream on the gpsimd queue in parallel with x on sync; the
            # mask/ones constants (not needed until attention) go last.
            bqkvt = constp.tile([128, 10], f32, name="bqkvt", tag="bqkvt")
            nc.sync.dma_start(bqkvt[:], bqkv[:])
            # tiny warmup source via memset - no DMA dependency, so the PE
            # warmup starts the moment the preamble ends.  Plain f32 (memset
            # can't write f32r): 4 cycles/row means each matmul streams 4x
            # longer, so fewer are needed for the HAM ramp.
            wsrc = constp.tile([128, 640], f32, name="wsrc", tag="wsrc")
            nc.vector.memset(wsrc[:], 0.25)
            wq = []
            for e in range(6):
                t = constp.tile([128, 384], iod, name=f"wq{e}", tag=f"wq{e}")
                nc.gpsimd.dma_start(t[:], wqkvT[e * 128:(e + 1) * 128, :])
                wq.append(t)
            wvn = []
            for e in range(6):
                t = constp.tile([128, vw_cols], iod, name=f"wv{e}",
                                tag=f"wv{e}")
                # scalar queue: in parallel with wq on gpsimd, so the V
                # pass of seq-block 0 isn't starved behind the q/k weights
                nc.scalar.dma_start(t[:], wvnat[e * 128:(e + 1) * 128, :])
                wvn.append(t)
            xt = []
            for e in range(6):
                t = constp.tile([128, S], iod, name=f"xt{e}", tag=f"xt{e}")
                xt.append(t)
            for e in range(6):
                nc.sync.dma_start(xt[e][:, 0:512],
                                  xT[0, e * 128:(e + 1) * 128, :])
            maskt = constp.tile([128, 896], mybir.dt.bfloat16, name="maskt",
                                tag="maskt")
            nc.scalar.dma_start(maskt[:], maskb[:])
            onest = constp.tile([65, 128], f32r, name="onest", tag="onest")
            nc.scalar.dma_start(onest[:], ones1[:])
            for sb in range(1, 4):
                for e in range(6):
                    nc.sync.dma_start(
                        xt[e][:, sb * 512:(sb + 1) * 512],
                        xT[sb, e * 128:(e + 1) * 128, :],
                    )
            woutp_t = constp.tile([128, E], f32r, name="woutp", tag="woutp")
            nc.gpsimd.dma_start(woutp_t[:], woutT[0:128, :])
            wout2_t = constp.tile([128, E], f32r, name="wout2", tag="wout2")
            nc.gpsimd.dma_start(wout2_t[:], woutT[128:256, :])

            # ---- PE warmup: lift HAM to 8/8 while DMAs stream ------------
            with tc.tile_pool(name="psW", bufs=1, space="PSUM") as psW:
                warm = psW.tile([128, 512], f32, name="warm", tag="warm")
                for i in range(6):
                    nc.tensor.matmul(warm[:], lhsT=wsrc[:, 0:128],
                                     rhs=wsrc[:, 128:640],
                                     start=(i == 0), stop=(i == 5))
                wsb = outp.tile([128, 512], f32, name="wsb", tag="wsb",
                                bufs=1)
                nc.vector.tensor_copy(wsb[:], warm[:])

            # ---- QKV projection into zero-padded per-head tiles ----------
            qz = []
            kz = []
            for h in range(3):
                tq = qkvp.tile([128, S], f32r, name=f"q{h}z", tag=f"q{h}z")
                tk = qkvp.tile([128, S], f32r, name=f"k{h}z", tag=f"k{h}z")
                qz.append(tq)
                kz.append(tk)
            # vaug[0]/vaug[2]: per 128-kpos chunk [V(64 cols) | ones]  ->
            #   attnV rows 0:63 = attnT, row 64 = softmax denominators.
            # vaug[1]: [ones | zeros(63) | V(64)] -> attnV row 0 = sums,
            #   rows 64:127 = attnT, so h1 lands at partition base 64 of
            #   at01 with no partition-shifting DMA.
            vaug = []
            vw = [65, 128, 65]
            for h in range(3):
                t = qkvp.tile([128, 16 * vw[h]], mybir.dt.bfloat16,
                              name=f"vaug{h}", tag=f"vaug{h}")
                vaug.append(t)
                r3 = t.rearrange("p (c w) -> p c w", w=vw[h])
                onescol = 64 if h != 1 else 0
                nc.vector.tensor_copy(
                    r3[:, :, onescol:onescol + 1],
                    bqkvt[:, 9:10].unsqueeze(1).broadcast_to([128, 16, 1]))
            nc.vector.tensor_copy(
                vaug[1].rearrange("p (c w) -> p c w", w=128)[:, :, 1:64],
                bqkvt[:, 8:9].unsqueeze(1).broadcast_to([128, 16, 63]))

            # zero the padded halves that no projection pass writes
            # (free-dim broadcast of an all-zero mask column)
            zsrc = bqkvt[64:128, 8:9].broadcast_to([64, S])
            nc.vector.tensor_copy(qz[2][64:128, :], zsrc)
            nc.vector.tensor_copy(kz[2][64:128, :], zsrc)

            # projection passes: (col offset, M, which tiles)
            passes = [(0, 128), (128, 128), (256, 64), (320, 64)]

            with tc.tile_pool(name="psA", bufs=2, space="PSUM") as psA:
                for sb in range(4):
                    ssl = slice(sb * 512, (sb + 1) * 512)
                    for ot in range(4):
                        off, M = passes[ot]
                        ps = psA.tile([M, 512], f32, name="qkvps",
                                      tag="qkvps")
                        for e in range(6):
                            nc.tensor.matmul(
                                ps[:],
                                lhsT=wq[e][:, off:off + M],
                                rhs=xt[e][:, ssl],
                                start=(e == 0),
                                stop=(e == 5),
                            )
                        # fused epilogue: out = ps*halfmask + bias via ACT
                        # Identity (bias cols pre-masked on host; cols 6/7
                        # are [1;0] / [0;1] partition masks).  ACT is idle
                        # during the projection phase and Identity is in
                        # every ACT table set, so this is free and keeps
                        # DVE clear for the attention phase.
                        mt = bqkvt[:, 6:7]
                        mb = bqkvt[:, 7:8]
                        if ot == 0:  # q0 top, q1 bottom
                            nc.scalar.activation(
                                qz[0][:, ssl], ps[:], AF.Identity,
                                bias=bqkvt[:, 0:1], scale=mt)
                            nc.scalar.activation(
                                qz[1][:, ssl], ps[:], AF.Identity,
                                bias=bqkvt[:, 1:2], scale=mb)
                        elif ot == 1:  # k0 top, k1 bottom
                            nc.scalar.activation(
                                kz[0][:, ssl], ps[:], AF.Identity,
                                bias=bqkvt[:, 2:3], scale=mt)
                            nc.scalar.activation(
                                kz[1][:, ssl], ps[:], AF.Identity,
                                bias=bqkvt[:, 3:4], scale=mb)
                        elif ot == 2:  # q2 (M=64; bottom zeroed up-front)
                            nc.scalar.activation(
                                qz[2][0:64, ssl], ps[:], AF.Identity,
                                bias=bqkvt[0:64, 4:5])
                        else:  # k2
                            nc.scalar.activation(
                                kz[2][0:64, ssl], ps[:], AF.Identity,
                                bias=bqkvt[0:64, 5:6])

                    # V natural-layout projection for this seq block
                    for st in range(sb * 4, sb * 4 + 4):
                        pv = psA.tile([128, vw_cols], f32, name="vnat",
                                      tag="vnat")
                        for e in range(6):
                            nc.tensor.matmul(
                                pv[:],
                                lhsT=xt[e][:, st * 128:(st + 1) * 128],
                                rhs=wvn[e][:],
                                start=(e == 0),
                                stop=(e == 5),
                            )
                        nc.vector.tensor_copy(
                            vaug[0][:, st * 65:st * 65 + 64], pv[:, 0:64])
                        nc.vector.tensor_copy(
                            vaug[1][:, st * 128 + 64:st * 128 + 128],
                            pv[:, 64:128])
                        nc.vector.tensor_copy(
                            vaug[2][:, st * 65:st * 65 + 64], pv[:, 128:192])

            # ---- attention + output projection ---------------------------
            at01 = qkvp.tile([128, S], f32r, name="at01", tag="at01")
            at2z = qkvp.tile([128, S], f32r, name="at2z", tag="at2z")

            # zero the padded bottom half of at2z once
            nc.vector.tensor_copy(at2z[64:128, :],
                                  bqkvt[64:128, 8:9].broadcast_to([64, S]))

            with tc.tile_pool(name="psB", bufs=1, space="PSUM") as psB:

                def attention(qb, after_first=None, descend=False):
                    nkc = 4 * (qb + 1)
                    qsl = slice(qb * 512, (qb + 1) * 512)
                    pa = []
                    for h in range(3):
                        t = psB.tile([65 if h != 1 else 128, 512], f32,
                                     name=f"pa{h}", tag="pa", bufs=3)
                        pa.append(t)

                    # kc2 group order: ascending (off-diagonal first).
                    # Measured in-process: diag-first and descending both
                    # lose ~10us - the diagonal chains compound with the
                    # boundary backlog instead of hiding in it.
                    if diag_first:
                        kc2s = ([4 * qb, 4 * qb + 2]
                                + list(range(0, 4 * qb, 2)))
                    else:
                        kc2s = list(range(0, nkc, 2))
                    if descend:
                        kc2s = list(range(0, nkc, 2))[::-1]
                    first_kc, last_kc = kc2s[0], kc2s[-1] + 1

                    def emit_attnv(h, kc2, wt):
                        for j in range(2):
                            kc = kc2 + j
                            # in descend mode diag chunks run full width
                            # (their dead cols are mask-zeroed) so the
                            # first emitted chunk's start covers all of pa
                            nw = 0 if descend else \
                                128 * max(0, kc - 4 * qb)
                            nc.tensor.matmul(
                                pa[h][:, nw:512],
                                lhsT=vaug[h].rearrange(
                                    "p (c w) -> p c w", w=vw[h])[:, kc, :],
                                rhs=wt[:, j * 512 + nw:(j + 1) * 512],
                                start=(kc == first_kc),
                                stop=(kc == last_kc),
                            )

                    # one-group software pipeline: emit group g's attnVs
                    # after group g+1's scores.  The previous q-block's
                    # broadcast+normalize (finish_a) goes FIRST: its
                    # inputs are long ready, so its matmuls fill the
                    # boundary bubble while the first score group waits
                    # for the previous block's exp backlog to drain.
                    if after_first is not None:
                        after_first[0]()
                    pend = []
                    for gi, kc2 in enumerate(kc2s):
                        cur = []
                        for h in range(3):
                            ps = psB.tile([128, 1024], f32, name="ps",
                                          tag="ps", bufs=2)
                            wt = workp.tile([128, 1024], mybir.dt.bfloat16,
                                            name="wt", tag="wt", bufs=12)
                            for j in range(2):
                                kc = kc2 + j
                                ksl = slice(kc * 128, (kc + 1) * 128)
                                # diagonal chunks: cols < 128c are fully
                                # masked - skip them in the score matmul
                                # (clamped so the moving dim stays >= 256,
                                # the f32r full-rate threshold).  descend
                                # mode computes full width instead (the
                                # mask zeroes the dead region, so the
                                # first chunk's start covers all of pa).
                                c = kc - 4 * qb
                                ms = 0 if (c <= 0 or descend) \
                                    else min(128 * c, 256)
                                nc.tensor.matmul(
                                    ps[:, j * 512 + ms:(j + 1) * 512],
                                    lhsT=kz[h][:, ksl],
                                    rhs=qz[h][:, qb * 512 + ms:
                                              (qb + 1) * 512],
                                    start=True, stop=True)
                            if kc2 - 4 * qb >= 0 and not descend:
                                # diag pair: exp only the live spans (cols
                                # < 128c of chunk c are fully masked;
                                # nothing below reads them)
                                e0 = 128 * (kc2 - 4 * qb)
                                e1 = e0 + 128
                                nc.scalar.activation(
                                    wt[:, e0:512], ps[:, e0:512], AF.Exp,
                                    scale=0.125)
                                nc.scalar.activation(
                                    wt[:, 512 + e1:1024],
                                    ps[:, 512 + e1:1024],
                                    AF.Exp, scale=0.125)
                            else:
                                nc.scalar.activation(wt[:], ps[:], AF.Exp,
                                                     scale=0.125)
                            for j in range(2):
                                kc = kc2 + j
                                if kc >= 4 * qb:
                                    c = kc - 4 * qb
                                    off = 384 - 128 * c
                                    # cols < 128c are fully masked: skip
                                    # them here and in the attnV (their
                                    # contribution is exactly zero); in
                                    # descend mode mask the full width
                                    # (cols < 384 of maskb are all-zero)
                                    m0 = 0 if descend else 128 * c
                                    nc.vector.tensor_mul(
                                        wt[:, j * 512 + m0:
                                           (j + 1) * 512],
                                        wt[:, j * 512 + m0:
                                           (j + 1) * 512],
                                        maskt[:, off + m0:off + 512])
                            cur.append((h, kc2, wt))
                        if gi == 0 and after_first is not None:
                            # splice the previous q-block's projection
                            # after the first score group: by now the
                            # normalize mults have landed
                            after_first[1]()
                            after_first = None
                        # two-group pipeline: each group's exps get a full
                        # extra group of slack before their attnVs issue,
                        # absorbing ACT jitter (the measured tail gaps
                        # were attnVs waiting on their own group's exp)
                        if len(pend) >= 3:
                            for args in pend.pop(0):
                                emit_attnv(*args)
                        pend.append(cur)
                    for grp in pend:
                        for args in grp:
                            emit_attnv(*args)
                    return pa

                def prep_normalize(qb, pa, fuse_finish=False):
                    # All pa readers live here (attnT copy + reciprocal of
                    # the sums row) so the accumulator banks free before
                    # the next q-block's attention needs them.  1/s is a
                    # single custom-DVE op (~51 ULP), then an ACT Copy
                    # casts it to f32r for the broadcast matmul - no ACT
                    # table switches, so Exp stays resident all kernel.
                    aus, rrs = {}, {}
                    for h in (0, 1, 2):
                        au = workp.tile([128, 512], f32, name="au",
                                        tag="au", bufs=3)
                        # copy the sums row along with the attn rows (same
                        # DVE cost - partitions run in parallel): custom
                        # DVE ops cannot read PSUM, so the reciprocal
                        # needs its input in SBUF.  Per-head fused chain in
                        # attnV completion order so the last q-block's
                        # normalize starts as soon as each head lands.
                        hsl = slice(0, 128) if h == 1 else slice(0, 65)
                        nc.vector.tensor_copy(au[hsl, :], pa[h][hsl, :])
                        aus[h] = au
                        rs = slice(0, 1) if h == 1 else slice(64, 65)
                        nrow = 65 if h != 1 else 1
                        rsb = workp.tile([65, 512], f32, name="rsb",
                                         tag="rsb", bufs=3)
                        # full-height recip: the custom-DVE op silently
                        # corrupts data when its AP starts at a nonzero
                        # partition, and extra partitions are free (the
                        # non-sums rows are garbage and never read)
                        nc.vector.reciprocal_approx_fast(
                            out=rsb[0:nrow, :], in_=au[0:nrow, :])
                        rr = workp.tile([65, 512], f32r, name="rr",
                                        tag="rr", bufs=3)
                        # f32->f32r cast on DVE: keeps the broadcast's
                        # dependency chain off the ACT queue, which still
                        # owes the current q-block's exps
                        nc.vector.tensor_copy(rr[rs, :], rsb[rs, :])
                        rrs[h] = rr
                        if fuse_finish:
                            # last q-block: broadcast + normalize per head
                            # immediately so the tail chain overlaps the
                            # remaining heads' attnV
                            qsl = slice(qb * 512, (qb + 1) * 512)
                            os_ = (onest[0:1, :] if h == 1
                                   else onest[64:65, :])
                            pbs = psB.tile([128, 512], f32, name="pbs",
                                           tag="py", bufs=1)
                            nc.tensor.matmul(pbs[:], lhsT=os_,
                                             rhs=rr[rs, :],
                                             start=True, stop=True)
                            if h == 0:
                                nc.vector.tensor_mul(
                                    at01[0:64, qsl], au[0:64, :],
                                    pbs[0:64, :])
                            elif h == 1:
                                nc.vector.tensor_mul(
                                    at01[64:128, qsl], au[64:128, :],
                                    pbs[64:128, :])
                            else:
                                nc.vector.tensor_mul(
                                    at2z[0:64, qsl], au[0:64, :],
                                    pbs[0:64, :])
                    return aus, rrs

                def finish_a(qb, aus, rrs):
                    # broadcast reciprocals (K=1 ones matmul), normalize.
                    # Emitted after the NEXT q-block's attention so the
                    # in-order PE stream never stalls on the DVE chain.
                    qsl = slice(qb * 512, (qb + 1) * 512)
                    for h in (1, 0, 2):
                        rs = slice(0, 1) if h == 1 else slice(64, 65)
                        os_ = onest[0:1, :] if h == 1 else onest[64:65, :]
                        pbs = psB.tile([128, 512], f32, name="pbs",
                                       tag="py", bufs=1)
                        nc.tensor.matmul(pbs[:], lhsT=os_,
                                         rhs=rrs[h][rs, :],
                                         start=True, stop=True)
                        if h == 1:
                            nc.vector.tensor_mul(
                                at01[64:128, qsl], aus[1][64:128, :],
                                pbs[64:128, :])
                        elif h == 0:
                            nc.vector.tensor_mul(
                                at01[0:64, qsl], aus[0][0:64, :],
                                pbs[0:64, :])
                        else:
                            nc.vector.tensor_mul(
                                at2z[0:64, qsl], aus[2][0:64, :],
                                pbs[0:64, :])
                def finish_b(qb, aus, rrs, last=False):
                    qsl = slice(qb * 512, (qb + 1) * 512)
                    for et in range(6):
                        esl = slice(et * 128, (et + 1) * 128)
                        # on the final q-block nothing contends for the
                        # score slots - alternate banks so the drain
                        # overlaps the next matmul pair
                        ptag = "ps" if (last and et % 2) else "py"
                        py = psB.tile([128, 512], f32, name="py", tag=ptag,
                                      bufs=2 if ptag == "ps" else 1)
                        nc.tensor.matmul(py[:], lhsT=woutp_t[:, esl],
                                         rhs=at01[:, qsl],
                                         start=True, stop=False)
                        nc.tensor.matmul(py[:], lhsT=wout2_t[:, esl],
                                         rhs=at2z[:, qsl],
                                         start=False, stop=True)
                        yt_t = outp.tile([128, 512], oud, name="yt",
                                         tag="yt")
                        if last:
                            # tail: split the PSUM->SBUF copies between
                            # DVE and the idle ACT, and the drain across
                            # four DMA queues
                            if et % 2 == 0:
                                nc.vector.tensor_copy(yt_t[:], py[:])
                            else:
                                nc.scalar.activation(yt_t[:], py[:],
                                                     AF.Copy)
                            eng = (nc.sync, nc.gpsimd, nc.scalar)[et % 3]
                            eng.dma_start(yT[qb, esl, :], yt_t[:])
                        else:
                            nc.vector.tensor_copy(yt_t[:], py[:])
                            nc.sync.dma_start(yT[qb, esl, :], yt_t[:])

                pending = None
                order = (0, 1, 2, 3)
                for qi, qb in enumerate(order):
                    cb = None
                    if pending:
                        cb = (lambda p=pending: finish_a(*p),
                              lambda p=pending: finish_b(*p))
                    pa = attention(qb, after_first=cb)
                    if qi == len(order) - 1:
                        aus, rrs = prep_normalize(qb, pa, fuse_finish=True)
                        finish_b(qb, aus, rrs, last=True)
                    else:
                        aus, rrs = prep_normalize(qb, pa)
                        pending = (qb, aus, rrs)

    nc.compile()
    return nc


BF16IO = bool(int(os.environ.get("KERNEL_BF16IO", "1")))


DIAG_FIRST = bool(int(os.environ.get("KERNEL_DIAG_FIRST", "0")))


def _get_nc(bf16io=None, diag_first=None):
    if bf16io is None:
        bf16io = BF16IO
    if diag_first is None:
        diag_first = DIAG_FIRST
    key = ("nc", bf16io, diag_first)
    if key not in _CACHE:
        _CACHE[key] = _build_nc(bf16io, diag_first)
    return _CACHE[key]


def _host_inputs(x, w_qkv, b_qkv, w_out, bf16io=None):
    """Build the 8 per-core input maps."""
    from ml_dtypes import bfloat16
    if bf16io is None:
        bf16io = BF16IO
    iodt = bfloat16 if bf16io else np.float32
    mbig = np.zeros((128, 896), np.float32)
    kp = np.arange(128)[:, None]
    t = np.arange(896)[None, :]
    mbig[kp <= t - 384] = 1.0
    mbig = mbig.astype(bfloat16)
    w_outT = np.ascontiguousarray(w_out.T)

    in_maps = []
    for core in range(NCORES):
        b, g = divmod(core, 4)
        base = 192 * g
        xTc = np.ascontiguousarray(
            x[b].T.reshape(E, 4, 512).transpose(1, 0, 2)).astype(iodt)
        q01 = w_qkv[base:base + 128]
        q2 = w_qkv[base + 128:base + 192]
        k01 = w_qkv[768 + base:768 + base + 128]
        k2 = w_qkv[768 + base + 128:768 + base + 192]
        wsl = np.concatenate([q01, k01, q2, k2], axis=0)       # [384, E]
        wqkvTc = np.ascontiguousarray(wsl.T).astype(iodt)
        if bf16io:
            wv = w_qkv[1536 + base:1536 + base + 192]
        else:
            wv = np.zeros((256, E), np.float32)
            wv[0:192] = w_qkv[1536 + base:1536 + base + 192]
        wvnatc = np.ascontiguousarray(wv.T).astype(iodt)
        bq = np.zeros((128, 10), np.float32)
        bq[:, 9] = 1.0
        bq[0:64, 0] = b_qkv[base:base + 64]               # q0 (top half)
        bq[64:128, 1] = b_qkv[base + 64:base + 128]       # q1 (bottom half)
        bq[0:64, 2] = b_qkv[768 + base:768 + base + 64]   # k0
        bq[64:128, 3] = b_qkv[768 + base + 64:768 + base + 128]  # k1
        bq[0:64, 4] = b_qkv[base + 128:base + 192]        # q2
        bq[0:64, 5] = b_qkv[768 + base + 128:768 + base + 192]   # k2
        bq[0:64, 6] = 1.0                                 # top-half mask
        bq[64:128, 7] = 1.0                               # bottom-half mask
        wo = np.zeros((256, E), np.float32)
        wo[0:192] = w_outT[base:base + 192]
        onesv = np.zeros((65, 128), np.float32)
        onesv[0, :] = 1.0
        onesv[64, :] = 1.0
        in_maps.append({
            "xT": xTc, "wqkvT": wqkvTc, "wvnat": wvnatc,
            "woutT": np.ascontiguousarray(wo), "maskb": mbig, "bqkv": bq,
            "ones1": onesv,
        })
    return in_maps


def _reference_numpy(x, mask, w_qkv, b_qkv, w_out, b_out):
    """Fallback for non-causal masks (never expected for this problem)."""
    b, s, _ = x.shape
    qkv = x @ w_qkv.T + b_qkv
    qkv = qkv.reshape(b, s, 3, H, D).transpose(2, 0, 3, 1, 4)
    q, k, v = qkv[0], qkv[1], qkv[2]
    scores = np.einsum("bhqd,bhkd->bhqk", q, k) * (D ** -0.5)
    scores = np.where(mask == 0, -np.inf, scores)
    scores -= scores.max(axis=-1, keepdims=True)
    w = np.exp(scores)
    w /= w.sum(axis=-1, keepdims=True)
    attn = np.einsum("bhqk,bhkd->bhqd", w, v)
    attn = attn.transpose(0, 2, 1, 3).reshape(b, s, E)
    return (attn @ w_out.T + b_out).astype(np.float32)


def kernel(x, mask, w_qkv, b_qkv, w_out, b_out):
    global LAST_RESULTS
    x = np.asarray(x, np.float32)
    mask = np.asarray(mask)
    w_qkv = np.asarray(w_qkv, np.float32)
    b_qkv = np.asarray(b_qkv, np.float32)
    w_out = np.asarray(w_out, np.float32)
    b_out = np.asarray(b_out, np.float32)

    # The device kernel bakes in causality; verify and fall back otherwise.
    m2 = np.asarray(mask).reshape(mask.shape[-2], mask.shape[-1])
    expect = np.tril(np.ones((S, S), m2.dtype))
    if m2.shape != (S, S) or not np.array_equal(m2, expect):
        return _reference_numpy(x, mask, w_qkv, b_qkv, w_out, b_out)

    from concourse.bass_utils import run_bass_kernel_spmd

    nc = _get_nc()
    in_maps = _host_inputs(x, w_qkv, b_qkv, w_out)
    trace = bool(int(os.environ.get("KERNEL_TRACE", "0")))
    kwargs = {}
    if trace:
        kwargs["trace"] = True
        kwargs["trace_cores"] = list(range(NCORES))
    res = run_bass_kernel_spmd(nc, in_maps, core_ids=list(range(NCORES)),
                               **kwargs)
    LAST_RESULTS = res

    # v-bias flows through the (normalized) attention as an additive
    # constant: y += w_out @ b_v.  Exact because softmax rows sum to 1.
    b_eff = b_out + w_out @ b_qkv[2 * E:]
    y = np.empty((B, S, E), np.float32)
    for b in range(B):
        acc = res.results[b * 4]["yT"].astype(np.float32).copy()
        for g in range(1, 4):
            acc += res.results[b * 4 + g]["yT"]
        # acc is [4, E, 512] (seq-block major) -> [S, E]
        y[b] = acc.transpose(0, 2, 1).reshape(S, E) + b_eff
    return y

